# revision 46
# baseline (speedup 1.0000x reference)
"""CPPN MLP (12 -> 32 -> 32 -> 32 -> 3, per-node activations) on 8 TRN2 cores.

Data-parallel over the pixel axis. Each core processes P_CORE pixels laid out
feature-major as 4 pixel-groups on SBUF partitions:
  rhs partition (12*g + i) holds feature i of pixel-group g  (layer-1 input)
  hidden state partition layout per layer: 4 groups x 32 nodes, nodes sorted
  [gauss | sin | tanh-class] across groups.

All matmul data (x, weights, hidden state h) is fp16: full-rate PE matmuls,
half the DMA bytes, 10-bit mantissa (~5e-4 relative) which the 2e-2 harness
gate easily absorbs.  PSUM accumulation stays fp32.

The tanh-class (tanh/sigmoid/identity) is handled by ONE Tanh pass over all
128 partitions with per-partition scale/bias operands plus host-side
algebraic folds into the next layer's weights:
  sigmoid(z) = 0.5*tanh(z/2) + 0.5          (stored tanh(z/2); affine folded)
  identity(z) = tanh(eps*z)/eps             (stored tanh(eps*z); 1/eps folded)

Sin and gauss rows are only ~1/4 of the partitions but a sub-range activation
op costs the same as a full-height one (cost ~ free-dim length).  So the main
Tanh pass writes those rows as the identity-eps encoding tanh(eps*(u+b)) ~=
eps*(u+b) (eps = 2^-9), they are DMA-GATHERED from several chunks' h tiles
into densely packed SBUF tiles (pack factor P_c = 128 // (4*n_c), capped),
the per-class op chains run once per packed tile (amortized ~P_c-fold), and
results are DMA-scattered back over the eps-junk rows of h:
  gauss(z) = exp(-(z+b)^2/2):  y = Square((0.5/eps)*enc) = ((u+b)/2)^2;
    t = Tanh(y);  gauss = 2/(1+t) - 1 via DVE reciprocal_approx_fast + one
    affine tensor_scalar.
  sin(z+b):  ADD_RANGE_WRAP wraps the encoding into [-eps*pi, eps*pi] (the
    wrap is linear so it works in eps-space; one period suffices since
    |z+b| < 3*pi), then Sin decodes with scale 1/eps.
Junk rows inside packed tiles (padding) flow through every op harmlessly and
are never scattered.  DMA issue is spread across sequencers (gathers, sin
scatters and x loads on GpSimd's software DGE; gauss scatters and output
stores on SP; none on Activation) so no sequencer's DIRECT2D issue cost
(~0.7us each) starves the ScalarE compute stream.
"""

import os
import sys

import numpy as np

_REPO = "/root/.axon_site/_ro/trn_rl_repo"
if _REPO not in sys.path and not os.path.isdir("/opt/trn_rl_repo"):
    sys.path.insert(0, _REPO)

import concourse.bacc as bacc
import concourse.bass as bass  # noqa: F401
import concourse.tile as tile
from concourse import mybir
from concourse.bass_utils import run_bass_kernel_spmd

# Pin the activation-function table to the single set containing every
# function this kernel uses ({Tanh, Square, Sin}).  Without this, bacc's
# greedy per-instruction set selection can alternate between sets and emit
# an ACT_TABLE_LOAD (~2.7us) per chunk.
_orig_get_tables = bacc.get_activation_tables


def _pinned_tables(arch):
    t = _orig_get_tables(arch)
    if "silu_and_others" in t:
        # act_func_set_id is the POSITION in act_info.json's set list, so
        # keep every entry (order intact) and just empty the others.
        return {name: (funcs if name == "silu_and_others" else set())
                for name, funcs in t.items()}
    return t


bacc.get_activation_tables = _pinned_tables

F32 = mybir.dt.float32
F32R = mybir.dt.float32r
F16 = mybir.dt.float16

P_TOTAL = 1024 * 1024
N_IN, H, N_OUT = 12, 32, 3
N_CORES = 8
P_CORE = P_TOTAL // N_CORES  # 131072
G = 4                        # pixel groups packed on partitions
PG = P_CORE // G             # 32768 pixels per group per core
CHUNK = 1024                 # pixels per group per chunk (2 PSUM banks)
MM_N = 512                   # matmul moving free dim (one PSUM bank)
ID_EPS = np.float32(2.0 ** -9)      # identity-via-tanh input scale
TWO_PI = float(2.0 * np.pi)
PI = float(np.pi)
P_CAP = 4                    # max chunks packed per class tile


def _pack_factor(n):
    """Chunks packed per class tile for a class with n nodes (4n rows)."""
    if n == 0:
        return 0
    return max(1, min(P_CAP, 128 // (4 * n)))


# class codes: 0 = gauss, 1 = sin, 2 = tanh-class (tanh/sigmoid/identity)
def _cls_of_act(a):
    return {4: 0, 3: 1}.get(int(a), 2)


def _sorted_layout(act):
    """Order the H nodes by [gauss | sin | rest]; return (perm, n_gauss, n_sin).
    perm[j] = original node index placed at sorted slot j."""
    cls = np.array([_cls_of_act(a) for a in act])
    perm = np.argsort(cls, kind="stable")
    return perm, int((cls == 0).sum()), int((cls == 1).sum())


class _Plan:
    """Host-side folded weights + per-layer layouts. All float64 math."""

    def __init__(self, bias_in, W1, b1, act1, W2, b2, act2, W3, b3, act3,
                 Wout, bout):
        layers = [(W1, b1, act1), (W2, b2, act2), (W3, b3, act3)]
        self.perms, self.ngauss, self.nsin = [], [], []
        self.lhsT = []          # device stationary matrices (np.float32)
        self.cols = []          # per-layer dict of [128] operand columns
        # incoming per-node output transform: h_true = alpha*stored + beta
        in_alpha = np.ones(N_IN, dtype=np.float64)
        in_beta = np.asarray(bias_in, dtype=np.float64)  # h0 = x + bias_in
        in_dim = N_IN
        in_layout = None  # for L1 the input layout is the fixed feature order

        for li, (W, b, act) in enumerate(layers):
            W = np.asarray(W, dtype=np.float64)
            b = np.asarray(b, dtype=np.float64)
            act = np.asarray(act)
            perm, ng, ns = _sorted_layout(act)
            self.perms.append(perm)
            self.ngauss.append(ng)
            self.nsin.append(ns)

            # effective weights / bias absorbing incoming transforms
            W_eff = W * in_alpha[:, None]                  # [in_dim, H]
            b_eff = b + in_beta @ W                        # [H]

            # device stationary: block diagonal over groups with node sort
            K = G * in_dim
            lt = np.zeros((K, 128), dtype=np.float64)
            for g in range(G):
                for j in range(H):
                    node = perm[j]
                    m = self._row(li, g, j)
                    if li == 0:
                        rows = np.arange(in_dim) + in_dim * g
                        lt[rows, m] = W_eff[:, node]
                    else:
                        for k_in in range(in_dim):
                            kpart = in_layout[g][k_in]
                            lt[kpart, m] = W_eff[k_in, node]
            self.lhsT.append(lt.astype(np.float32))

            # operand columns.  Main tanh pass: per-partition scale/bias.
            # Packed class tiles: per-partition bias patterns replicated
            # once per packed block.
            tanh_scale = np.zeros(128, dtype=np.float64)
            tanh_bias = np.zeros(128, dtype=np.float64)
            out_alpha = np.ones(H, dtype=np.float64)
            out_beta = np.zeros(H, dtype=np.float64)
            for j in range(H):
                node = perm[j]
                a = int(act[node])
                be = b_eff[node]
                for g in range(G):
                    m = self._row(li, g, j)
                    if a == 1:        # tanh
                        tanh_scale[m] = 1.0
                        tanh_bias[m] = be
                    elif a == 2:      # sigmoid -> tanh(u/2)
                        tanh_scale[m] = 0.5
                        tanh_bias[m] = 0.5 * be
                    else:
                        # identity nodes AND the sin/gauss rows: the main
                        # tanh pass writes the identity-eps encoding
                        # tanh(eps*(u+b)) ~= eps*(u+b), which for sin/gauss
                        # is the value the packed chains gather from h
                        # (DMA cannot read PSUM).
                        tanh_scale[m] = float(ID_EPS)
                        tanh_bias[m] = float(ID_EPS) * be
                if a == 1:
                    out_alpha[node], out_beta[node] = 1.0, 0.0
                elif a == 2:
                    out_alpha[node], out_beta[node] = 0.5, 0.5
                elif a == 0:
                    out_alpha[node], out_beta[node] = 1.0 / float(ID_EPS), 0.0
                else:                 # sin / gauss: stored value is exact
                    out_alpha[node], out_beta[node] = 1.0, 0.0
            self.cols.append({
                "tanh_scale": tanh_scale, "tanh_bias": tanh_bias,
            })

            # next layer's incoming transform, in SORTED node order per device
            # partition -> but folds are per node; store per-node arrays and
            # the partition layout for the next lhsT build.
            in_alpha = out_alpha
            in_beta = out_beta
            in_dim = H
            # partition index of (g, sorted-slot j) for this layer's output
            in_layout = [[self._row(li, g, j) for j in range(H)]
                         for g in range(G)]
            # reorder alpha/beta to sorted-slot order for the next W_eff
            in_alpha = out_alpha[perm]
            in_beta = out_beta[perm]
            # next layer's W rows must be permuted accordingly
            if li < 2:
                layers[li + 1] = (np.asarray(layers[li + 1][0])[perm, :],
                                  layers[li + 1][1], layers[li + 1][2])
            else:
                self._wout_perm = perm

        # output layer
        Wo = np.asarray(Wout, dtype=np.float64)[self._wout_perm, :]
        bo = np.asarray(bout, dtype=np.float64)
        Wo_eff = Wo * in_alpha[:, None]
        bo_eff = bo + in_beta @ Wo
        lt = np.zeros((128, 32), dtype=np.float64)
        for g in range(G):
            for j in range(H):
                kpart = in_layout[g][j]
                for o in range(N_OUT):
                    lt[kpart, 3 * g + o] = Wo_eff[j, o]
        self.lhsT_out = lt.astype(np.float32)
        out_bias = np.zeros(128, dtype=np.float64)
        for q in range(4):
            for g in range(G):
                for o in range(N_OUT):
                    out_bias[32 * q + 3 * g + o] = bo_eff[o]
        self.out_bias = out_bias

        # pack all operand columns into one [128, 32] block
        colblk = np.zeros((128, 32), dtype=np.float64)
        for li in range(3):
            c = self.cols[li]
            colblk[:, 8 * li + 0] = c["tanh_scale"]
            colblk[:, 8 * li + 1] = c["tanh_bias"]
        colblk[:, 24] = self.out_bias
        self.colblk = colblk.astype(np.float32)

    @staticmethod
    def _row(li, g, j):
        """Device partition of sorted-slot j, group g (layer output layout).
        Rows are class-sorted ACROSS groups: slot j occupies partitions
        4*j + g."""
        return 4 * j + g


def _build_program(ngauss, nsin, p_core=P_CORE, chunk=CHUNK,
                   use_fp32r=True):
    """Build the bass module. Program structure depends only on the per-layer
    (n_gauss, n_sin) counts, not on weight values."""
    pg = p_core // G
    nchunk = pg // chunk
    nhalf = chunk // MM_N
    assert chunk % MM_N == 0 and pg % chunk == 0

    pfg = [_pack_factor(n) for n in ngauss]   # gauss pack factor per layer
    pfs = [_pack_factor(n) for n in nsin]     # sin pack factor per layer
    # per-layer group phase offsets: stagger chain firing across layers so
    # the packed chains don't all burst on the same chunks
    goff = [0, 0, 0]
    soff = [0, 0, 0]
    # emission skew between layers: a chunk's h completes only when its
    # packed group completes (dataflow-enforced; skew is a priority hint)
    skew = [0, 4, 8, 12]
    total_skew = skew[3] + 1

    nc = bacc.Bacc("TRN2", target_bir_lowering=False, debug=False,
                   num_devices=N_CORES)
    xT = nc.dram_tensor("xT", [G * N_IN, pg], F16, kind="ExternalInput").ap()
    wst = nc.dram_tensor("wst", [128, 416], F16, kind="ExternalInput").ap()
    cst = nc.dram_tensor("cst", [128, 64], F32, kind="ExternalInput").ap()
    yT = nc.dram_tensor("yT", [12, pg], F32, kind="ExternalOutput").ap()

    with tile.TileContext(nc) as tc:
        cpool = tc.alloc_tile_pool(name="consts", bufs=1)
        wst_t = cpool.tile([128, 416], F16, tag="wst")
        cc_t = cpool.tile([128, 32], F32, tag="cc")
        nc.sync.dma_start(out=wst_t[:], in_=wst[:, 0:416])
        nc.sync.dma_start(out=cc_t[:], in_=cst[:, 0:32])
        w1_t = wst_t[:, 0:128]
        w2_t = wst_t[:, 128:256]
        w3_t = wst_t[:, 256:384]
        wo_t = wst_t[:, 384:416]
        col_t = cc_t[:, 0:32]

        xpool = tc.alloc_tile_pool(name="xin", bufs=6)
        hpool = tc.alloc_tile_pool(name="h", bufs=14)
        gpool = tc.alloc_tile_pool(name="gpack", bufs=3)   # packed gauss z
        spool = tc.alloc_tile_pool(name="spack", bufs=3)   # packed sin z
        scpool = tc.alloc_tile_pool(name="scr", bufs=3)    # packed scratch
        rpool = tc.alloc_tile_pool(name="res", bufs=4)     # packed results
        opool = tc.alloc_tile_pool(name="osb", bufs=2)
        ppool = tc.alloc_tile_pool(name="psum", bufs=3, space="PSUM")
        oppool = tc.alloc_tile_pool(name="psum_o", bufs=2, space="PSUM")

        w_tiles = [w1_t, w2_t, w3_t]
        h_live = {}     # (chunk, li) -> produced tile (li 0 == x input)
        pso_live = {}   # chunk-pair -> psum_o tile
        gpk = {}        # li -> current packed gauss tile
        spk = {}        # li -> current packed sin tile
        gst = {li: {"blk": 0, "chunks": []} for li in range(3)}
        sst = {li: {"blk": 0, "chunks": []} for li in range(3)}

        hdt = F16 if use_fp32r else F32

        def emit_load(c):
            x_t = xpool.tile([G * N_IN, chunk], F16, tag="x")
            nc.sync.dma_start(
                out=x_t[:], in_=xT[:, c * chunk:(c + 1) * chunk])
            h_live[(c, 0)] = x_t

        def emit_main(c, li):
            """Main matmuls + full-height tanh pass + class gathers."""
            h_prev = h_live.pop((c, li))
            ng, ns = ngauss[li], nsin[li]
            kdim = G * N_IN if li == 0 else 128
            ps = ppool.tile([128, chunk], F32, tag="pre")
            wt = w_tiles[li]
            for hh in range(nhalf):
                sl = slice(hh * MM_N, (hh + 1) * MM_N)
                nc.tensor.matmul(
                    ps[:, sl],
                    wt[0:kdim, :],
                    h_prev[0:kdim, sl],
                    start=True, stop=True,
                )
            h = hpool.tile([128, chunk], hdt, tag=f"h{li}")
            cb = 8 * li
            # tanh-class pass over all 128 rows (junk on gauss/sin rows,
            # overwritten by the scatters)
            nc.scalar.activation(
                h[:], ps[:], mybir.ActivationFunctionType.Tanh,
                bias=col_t[:, cb + 1:cb + 2],
                scale=col_t[:, cb + 0:cb + 1],
            )
            # gather class rows (eps-encoded u+b) from h into packed tiles
            if ng > 0:
                st = gst[li]
                if st["blk"] == 0:
                    gpk[li] = gpool.tile([128, chunk], hdt, tag=f"gz{li}",
                                         name=f"gz{li}")
                eng = nc.sync if li < 2 else nc.gpsimd
                eng.dma_start(
                    out=gpk[li][st["blk"] * 4 * ng:
                                (st["blk"] + 1) * 4 * ng, :],
                    in_=h[0:4 * ng, :])
                st["blk"] += 1
                st["chunks"].append(c)
            if ns > 0:
                st = sst[li]
                if st["blk"] == 0:
                    spk[li] = spool.tile([128, chunk], hdt, tag=f"sz{li}",
                                         name=f"sz{li}")
                nc.gpsimd.dma_start(
                    out=spk[li][st["blk"] * 4 * ns:
                                (st["blk"] + 1) * 4 * ns, :],
                    in_=h[4 * ng:4 * (ng + ns), :])
                st["blk"] += 1
                st["chunks"].append(c)
            h_live[(c, li + 1)] = h

        def emit_gauss_pack(li):
            """Packed gauss chain for the accumulated group."""
            ng = ngauss[li]
            st = gst[li]
            nblk = st["blk"]
            R = 4 * ng * nblk
            gz = gpk[li]
            # y = Square((0.5/eps) * epsenc) = ((u+b)/2)^2 (in-place, fp16:
            # y is decoded so its magnitude is O(1) - fp16-safe)
            nc.scalar.activation(
                gz[0:R, :], gz[0:R, :],
                mybir.ActivationFunctionType.Square,
                scale=float(0.5 / ID_EPS),
            )
            # t = tanh(y)
            t_t = scpool.tile([128, chunk], F32, tag="gt")
            nc.scalar.activation(
                t_t[0:R, :], gz[0:R, :], mybir.ActivationFunctionType.Tanh)
            # den = 1 + t ;  r = 1/den ;  out = 2r - 1 = exp(-(z+b)^2/2)
            den_t = scpool.tile([128, chunk], F32, tag="gd")
            nc.vector.tensor_scalar(
                den_t[0:R, :], t_t[0:R, :], 1.0, None, mybir.AluOpType.add)
            rin_t = scpool.tile([128, chunk], F32, tag="gr")
            nc.vector.reciprocal_approx_fast(rin_t[0:R, :], den_t[0:R, :])
            g_r = rpool.tile([128, chunk], hdt, tag="go")
            nc.vector.tensor_scalar(
                g_r[0:R, :], rin_t[0:R, :], 2.0, -1.0,
                mybir.AluOpType.mult, mybir.AluOpType.add)
            # scatter back into each chunk's h rows [0 : 4ng)
            for blk, c in enumerate(st["chunks"]):
                nc.sync.dma_start(
                    out=h_live[(c, li + 1)][0:4 * ng, :],
                    in_=g_r[blk * 4 * ng:(blk + 1) * 4 * ng, :])
            st["blk"] = 0
            st["chunks"] = []

        def emit_sin_pack(li):
            """Packed sin chain for the accumulated group."""
            ng, ns = ngauss[li], nsin[li]
            st = sst[li]
            nblk = st["blk"]
            R = 4 * ns * nblk
            sz = spk[li]
            # wrap the eps-encoded u+b into [-eps*pi, eps*pi] (all linear,
            # so the wrap works in eps-space), then decode in Sin's scale.
            m_t = scpool.tile([128, chunk], F32, tag="sm")
            nc.vector.add_range_wrap(
                m_t[0:R, :], sz[0:R, :],
                0.0, float(ID_EPS) * PI, float(ID_EPS) * TWO_PI)
            s_r = rpool.tile([128, chunk], hdt, tag="so")
            nc.scalar.activation(
                s_r[0:R, :], m_t[0:R, :], mybir.ActivationFunctionType.Sin,
                scale=float(1.0 / ID_EPS))
            # scatter back into each chunk's h rows [4ng : 4(ng+ns))
            for blk, c in enumerate(st["chunks"]):
                nc.gpsimd.dma_start(
                    out=h_live[(c, li + 1)][4 * ng:4 * (ng + ns), :],
                    in_=s_r[blk * 4 * ns:(blk + 1) * 4 * ns, :])
            st["blk"] = 0
            st["chunks"] = []

        def emit_layer(c, li):
            emit_main(c, li)
            if ngauss[li] > 0 and ((c + goff[li]) % pfg[li] == pfg[li] - 1
                                   or c == nchunk - 1):
                emit_gauss_pack(li)
            if nsin[li] > 0 and ((c + soff[li]) % pfs[li] == pfs[li] - 1
                                 or c == nchunk - 1):
                emit_sin_pack(li)

        def emit_out(c):
            # output layer: quadrant-packed [12,512] matmuls
            h_prev = h_live.pop((c, 3))
            q0 = 2 * (c % 2)
            if q0 == 0:
                pso_live[c // 2] = oppool.tile([128, MM_N], F32, tag="preo",
                                               name="pso")
            pso = pso_live[c // 2]
            for hh in range(nhalf):
                q = q0 + hh
                nc.tensor.matmul(
                    pso[32 * q:32 * q + 32, :],
                    wo_t,
                    h_prev[:, hh * MM_N:(hh + 1) * MM_N],
                    start=True, stop=True,
                    tile_position=(0, 32 * q),
                )
            if q0 == 2:
                pso_live.pop(c // 2)
                osb = opool.tile([128, MM_N], F32, tag="osb")
                nc.scalar.activation(
                    osb[:], pso[:],
                    mybir.ActivationFunctionType.Tanh,
                    bias=col_t[:, 24:25],
                )
                base = (c - 1) * chunk
                for q in range(4):
                    nc.sync.dma_start(
                        out=yT[:, base + q * MM_N: base + (q + 1) * MM_N],
                        in_=osb[32 * q:32 * q + 12, :])

        # Software-pipelined emission with per-layer skew covering the packed
        # group latency: layer li+1 of chunk c is emitted only after layer
        # li's packed groups containing c have been emitted.
        assert nchunk % 2 == 0
        for t in range(nchunk + total_skew):
            if t < nchunk:
                emit_load(t)
                emit_layer(t, 0)
            if skew[1] <= t and t - skew[1] < nchunk:
                emit_layer(t - skew[1], 1)
            if skew[2] <= t and t - skew[2] < nchunk:
                emit_layer(t - skew[2], 2)
            if skew[3] <= t and t - skew[3] < nchunk:
                emit_out(t - skew[3])

        for p in (oppool, ppool, opool, rpool, scpool, spool,
                  gpool, hpool, xpool, cpool):
            p.release()

    nc.compile()
    return nc


_PROGRAM_CACHE = {}


def _get_program(ngauss, nsin, p_core=P_CORE, chunk=CHUNK, use_fp32r=True):
    key = (tuple(ngauss), tuple(nsin), p_core, chunk, use_fp32r)
    if key not in _PROGRAM_CACHE:
        _PROGRAM_CACHE[key] = _build_program(ngauss, nsin, p_core, chunk,
                                             use_fp32r=use_fp32r)
    return _PROGRAM_CACHE[key]


def make_in_maps(inputs, plan, p_core=P_CORE, n_cores=N_CORES):
    """Shard + transpose the pixel data; replicate constants."""
    x = np.ascontiguousarray(np.asarray(inputs["inputs"], dtype=np.float32))
    pg = p_core // G
    in_maps = []
    for core in range(n_cores):
        xc = x[core * p_core:(core + 1) * p_core]          # [p_core, 12]
        xg = xc.reshape(G, pg, N_IN)                        # [G, pg, 12]
        xT = np.ascontiguousarray(xg.transpose(0, 2, 1)
                                  .reshape(G * N_IN, pg)
                                  .astype(np.float16))      # [48, pg]
        wst = np.zeros((128, 416), dtype=np.float16)
        wst[0:G * N_IN, 0:128] = plan.lhsT[0]
        wst[:, 128:256] = plan.lhsT[1]
        wst[:, 256:384] = plan.lhsT[2]
        wst[:, 384:416] = plan.lhsT_out
        cst = np.zeros((128, 64), dtype=np.float32)
        cst[:, 0:32] = plan.colblk
        in_maps.append({"xT": xT, "wst": wst, "cst": cst})
    return in_maps


def assemble_output(results, p_core=P_CORE, n_cores=N_CORES):
    pg = p_core // G
    out = np.empty((p_core * n_cores, N_OUT), dtype=np.float32)
    for core in range(n_cores):
        yT = results[core]["yT"]                            # [12, pg]
        yc = yT.reshape(G, N_OUT, pg).transpose(0, 2, 1)    # [G, pg, 3]
        out[core * p_core:(core + 1) * p_core] = yc.reshape(p_core, N_OUT)
    return out


def make_plan(inputs):
    return _Plan(
        inputs["bias_in"], inputs["W1"], inputs["b1"], inputs["act1"],
        inputs["W2"], inputs["b2"], inputs["act2"],
        inputs["W3"], inputs["b3"], inputs["act3"],
        inputs["Wout"], inputs["bout"])


def run(inputs, trace=False, use_fp32r=True, **spmd_kwargs):
    plan = make_plan(inputs)
    nc = _get_program(plan.ngauss, plan.nsin, use_fp32r=use_fp32r)
    in_maps = make_in_maps(inputs, plan)
    res = run_bass_kernel_spmd(nc, in_maps, list(range(N_CORES)),
                               trace=trace, **spmd_kwargs)
    return assemble_output(res.results), res


def kernel(**inputs) -> np.ndarray:
    out, _ = run(inputs, trace=False)
    return out


# revision 48
# speedup vs baseline: 1.0017x; 1.0017x over previous
"""CPPN MLP (12 -> 32 -> 32 -> 32 -> 3, per-node activations) on 8 TRN2 cores.

Data-parallel over the pixel axis. Each core processes P_CORE pixels laid out
feature-major as 4 pixel-groups on SBUF partitions:
  rhs partition (12*g + i) holds feature i of pixel-group g  (layer-1 input)
  hidden state partition layout per layer: 4 groups x 32 nodes, nodes sorted
  [gauss | sin | tanh-class] across groups.

All matmul data (x, weights, hidden state h) is fp16: full-rate PE matmuls,
half the DMA bytes, 10-bit mantissa (~5e-4 relative) which the 2e-2 harness
gate easily absorbs.  PSUM accumulation stays fp32.

The tanh-class (tanh/sigmoid/identity) is handled by ONE Tanh pass over all
128 partitions with per-partition scale/bias operands plus host-side
algebraic folds into the next layer's weights:
  sigmoid(z) = 0.5*tanh(z/2) + 0.5          (stored tanh(z/2); affine folded)
  identity(z) = tanh(eps*z)/eps             (stored tanh(eps*z); 1/eps folded)

Sin and gauss rows are only ~1/4 of the partitions but a sub-range activation
op costs the same as a full-height one (cost ~ free-dim length).  So the main
Tanh pass writes those rows as the identity-eps encoding tanh(eps*(u+b)) ~=
eps*(u+b) (eps = 2^-9), they are DMA-GATHERED from several chunks' h tiles
into densely packed SBUF tiles (pack factor P_c = 128 // (4*n_c), capped),
the per-class op chains run once per packed tile (amortized ~P_c-fold), and
results are DMA-scattered back over the eps-junk rows of h:
  gauss(z) = exp(-(z+b)^2/2):  y = Square((0.5/eps)*enc) = ((u+b)/2)^2;
    t = Tanh(y);  gauss = 2/(1+t) - 1 via DVE reciprocal_approx_fast + one
    affine tensor_scalar.
  sin(z+b):  ADD_RANGE_WRAP wraps the encoding into [-eps*pi, eps*pi] (the
    wrap is linear so it works in eps-space; one period suffices since
    |z+b| < 3*pi), then Sin decodes with scale 1/eps.
Junk rows inside packed tiles (padding) flow through every op harmlessly and
are never scattered.  DMA issue is spread across sequencers (gathers, sin
scatters and x loads on GpSimd's software DGE; gauss scatters and output
stores on SP; none on Activation) so no sequencer's DIRECT2D issue cost
(~0.7us each) starves the ScalarE compute stream.
"""

import os
import sys

import numpy as np

_REPO = "/root/.axon_site/_ro/trn_rl_repo"
if _REPO not in sys.path and not os.path.isdir("/opt/trn_rl_repo"):
    sys.path.insert(0, _REPO)

import concourse.bacc as bacc
import concourse.bass as bass  # noqa: F401
import concourse.tile as tile
from concourse import mybir
from concourse.bass_utils import run_bass_kernel_spmd

# Pin the activation-function table to the single set containing every
# function this kernel uses ({Tanh, Square, Sin}).  Without this, bacc's
# greedy per-instruction set selection can alternate between sets and emit
# an ACT_TABLE_LOAD (~2.7us) per chunk.
_orig_get_tables = bacc.get_activation_tables


def _pinned_tables(arch):
    t = _orig_get_tables(arch)
    if "silu_and_others" in t:
        # act_func_set_id is the POSITION in act_info.json's set list, so
        # keep every entry (order intact) and just empty the others.
        return {name: (funcs if name == "silu_and_others" else set())
                for name, funcs in t.items()}
    return t


bacc.get_activation_tables = _pinned_tables

F32 = mybir.dt.float32
F32R = mybir.dt.float32r
F16 = mybir.dt.float16

P_TOTAL = 1024 * 1024
N_IN, H, N_OUT = 12, 32, 3
N_CORES = 8
P_CORE = P_TOTAL // N_CORES  # 131072
G = 4                        # pixel groups packed on partitions
PG = P_CORE // G             # 32768 pixels per group per core
CHUNK = 1024                 # pixels per group per chunk (2 PSUM banks)
MM_N = 512                   # matmul moving free dim (one PSUM bank)
ID_EPS = np.float32(2.0 ** -9)      # identity-via-tanh input scale
TWO_PI = float(2.0 * np.pi)
PI = float(np.pi)
P_CAP = 4                    # max chunks packed per class tile


def _pack_factor(n):
    """Chunks packed per class tile for a class with n nodes (4n rows)."""
    if n == 0:
        return 0
    return max(1, min(P_CAP, 128 // (4 * n)))


# class codes: 0 = gauss, 1 = sin, 2 = tanh-class (tanh/sigmoid/identity)
def _cls_of_act(a):
    return {4: 0, 3: 1}.get(int(a), 2)


def _sorted_layout(act):
    """Order the H nodes by [gauss | sin | rest]; return (perm, n_gauss, n_sin).
    perm[j] = original node index placed at sorted slot j."""
    cls = np.array([_cls_of_act(a) for a in act])
    perm = np.argsort(cls, kind="stable")
    return perm, int((cls == 0).sum()), int((cls == 1).sum())


class _Plan:
    """Host-side folded weights + per-layer layouts. All float64 math."""

    def __init__(self, bias_in, W1, b1, act1, W2, b2, act2, W3, b3, act3,
                 Wout, bout):
        layers = [(W1, b1, act1), (W2, b2, act2), (W3, b3, act3)]
        self.perms, self.ngauss, self.nsin = [], [], []
        self.lhsT = []          # device stationary matrices (np.float32)
        self.cols = []          # per-layer dict of [128] operand columns
        # incoming per-node output transform: h_true = alpha*stored + beta
        in_alpha = np.ones(N_IN, dtype=np.float64)
        in_beta = np.asarray(bias_in, dtype=np.float64)  # h0 = x + bias_in
        in_dim = N_IN
        in_layout = None  # for L1 the input layout is the fixed feature order

        for li, (W, b, act) in enumerate(layers):
            W = np.asarray(W, dtype=np.float64)
            b = np.asarray(b, dtype=np.float64)
            act = np.asarray(act)
            perm, ng, ns = _sorted_layout(act)
            self.perms.append(perm)
            self.ngauss.append(ng)
            self.nsin.append(ns)

            # effective weights / bias absorbing incoming transforms
            W_eff = W * in_alpha[:, None]                  # [in_dim, H]
            b_eff = b + in_beta @ W                        # [H]

            # device stationary: block diagonal over groups with node sort
            K = G * in_dim
            lt = np.zeros((K, 128), dtype=np.float64)
            for g in range(G):
                for j in range(H):
                    node = perm[j]
                    m = self._row(li, g, j)
                    if li == 0:
                        rows = np.arange(in_dim) + in_dim * g
                        lt[rows, m] = W_eff[:, node]
                    else:
                        for k_in in range(in_dim):
                            kpart = in_layout[g][k_in]
                            lt[kpart, m] = W_eff[k_in, node]
            self.lhsT.append(lt.astype(np.float32))

            # operand columns.  Main tanh pass: per-partition scale/bias.
            # Packed class tiles: per-partition bias patterns replicated
            # once per packed block.
            tanh_scale = np.zeros(128, dtype=np.float64)
            tanh_bias = np.zeros(128, dtype=np.float64)
            out_alpha = np.ones(H, dtype=np.float64)
            out_beta = np.zeros(H, dtype=np.float64)
            for j in range(H):
                node = perm[j]
                a = int(act[node])
                be = b_eff[node]
                for g in range(G):
                    m = self._row(li, g, j)
                    if a == 1:        # tanh
                        tanh_scale[m] = 1.0
                        tanh_bias[m] = be
                    elif a == 2:      # sigmoid -> tanh(u/2)
                        tanh_scale[m] = 0.5
                        tanh_bias[m] = 0.5 * be
                    else:
                        # identity nodes AND the sin/gauss rows: the main
                        # tanh pass writes the identity-eps encoding
                        # tanh(eps*(u+b)) ~= eps*(u+b), which for sin/gauss
                        # is the value the packed chains gather from h
                        # (DMA cannot read PSUM).
                        tanh_scale[m] = float(ID_EPS)
                        tanh_bias[m] = float(ID_EPS) * be
                if a == 1:
                    out_alpha[node], out_beta[node] = 1.0, 0.0
                elif a == 2:
                    out_alpha[node], out_beta[node] = 0.5, 0.5
                elif a == 0:
                    out_alpha[node], out_beta[node] = 1.0 / float(ID_EPS), 0.0
                else:                 # sin / gauss: stored value is exact
                    out_alpha[node], out_beta[node] = 1.0, 0.0
            self.cols.append({
                "tanh_scale": tanh_scale, "tanh_bias": tanh_bias,
            })

            # next layer's incoming transform, in SORTED node order per device
            # partition -> but folds are per node; store per-node arrays and
            # the partition layout for the next lhsT build.
            in_alpha = out_alpha
            in_beta = out_beta
            in_dim = H
            # partition index of (g, sorted-slot j) for this layer's output
            in_layout = [[self._row(li, g, j) for j in range(H)]
                         for g in range(G)]
            # reorder alpha/beta to sorted-slot order for the next W_eff
            in_alpha = out_alpha[perm]
            in_beta = out_beta[perm]
            # next layer's W rows must be permuted accordingly
            if li < 2:
                layers[li + 1] = (np.asarray(layers[li + 1][0])[perm, :],
                                  layers[li + 1][1], layers[li + 1][2])
            else:
                self._wout_perm = perm

        # output layer
        Wo = np.asarray(Wout, dtype=np.float64)[self._wout_perm, :]
        bo = np.asarray(bout, dtype=np.float64)
        Wo_eff = Wo * in_alpha[:, None]
        bo_eff = bo + in_beta @ Wo
        lt = np.zeros((128, 32), dtype=np.float64)
        for g in range(G):
            for j in range(H):
                kpart = in_layout[g][j]
                for o in range(N_OUT):
                    lt[kpart, 3 * g + o] = Wo_eff[j, o]
        self.lhsT_out = lt.astype(np.float32)
        out_bias = np.zeros(128, dtype=np.float64)
        for q in range(4):
            for g in range(G):
                for o in range(N_OUT):
                    out_bias[32 * q + 3 * g + o] = bo_eff[o]
        self.out_bias = out_bias

        # pack all operand columns into one [128, 32] block
        colblk = np.zeros((128, 32), dtype=np.float64)
        for li in range(3):
            c = self.cols[li]
            colblk[:, 8 * li + 0] = c["tanh_scale"]
            colblk[:, 8 * li + 1] = c["tanh_bias"]
        colblk[:, 24] = self.out_bias
        self.colblk = colblk.astype(np.float32)

    @staticmethod
    def _row(li, g, j):
        """Device partition of sorted-slot j, group g (layer output layout).
        Rows are class-sorted ACROSS groups: slot j occupies partitions
        4*j + g."""
        return 4 * j + g


def _build_program(ngauss, nsin, p_core=P_CORE, chunk=CHUNK,
                   use_fp32r=True):
    """Build the bass module. Program structure depends only on the per-layer
    (n_gauss, n_sin) counts, not on weight values."""
    pg = p_core // G
    nchunk = pg // chunk
    nhalf = chunk // MM_N
    assert chunk % MM_N == 0 and pg % chunk == 0

    pfg = [_pack_factor(n) for n in ngauss]   # gauss pack factor per layer
    pfs = [_pack_factor(n) for n in nsin]     # sin pack factor per layer
    # per-layer group phase offsets: stagger chain firing across layers so
    # the packed chains don't all burst on the same chunks
    goff = [0, 0, 0]
    soff = [0, 0, 0]
    # emission skew between layers: a chunk's h completes only when its
    # packed group completes (dataflow-enforced; skew is a priority hint)
    skew = [0, 4, 8, 12]
    total_skew = skew[3] + 1

    nc = bacc.Bacc("TRN2", target_bir_lowering=False, debug=False,
                   num_devices=N_CORES)
    xT = nc.dram_tensor("xT", [G * N_IN, pg], F16, kind="ExternalInput").ap()
    wst = nc.dram_tensor("wst", [128, 416], F16, kind="ExternalInput").ap()
    cst = nc.dram_tensor("cst", [128, 64], F32, kind="ExternalInput").ap()
    yT = nc.dram_tensor("yT", [12, pg], F32, kind="ExternalOutput").ap()

    with tile.TileContext(nc) as tc:
        cpool = tc.alloc_tile_pool(name="consts", bufs=1)
        wst_t = cpool.tile([128, 416], F16, tag="wst")
        cc_t = cpool.tile([128, 32], F32, tag="cc")
        nc.sync.dma_start(out=wst_t[:], in_=wst[:, 0:416])
        nc.sync.dma_start(out=cc_t[:], in_=cst[:, 0:32])
        w1_t = wst_t[:, 0:128]
        w2_t = wst_t[:, 128:256]
        w3_t = wst_t[:, 256:384]
        wo_t = wst_t[:, 384:416]
        col_t = cc_t[:, 0:32]

        xpool = tc.alloc_tile_pool(name="xin", bufs=6)
        hpool = tc.alloc_tile_pool(name="h", bufs=14)
        gpool = tc.alloc_tile_pool(name="gpack", bufs=3)   # packed gauss z
        spool = tc.alloc_tile_pool(name="spack", bufs=3)   # packed sin z
        scpool = tc.alloc_tile_pool(name="scr", bufs=3)    # packed scratch
        rpool = tc.alloc_tile_pool(name="res", bufs=4)     # packed results
        opool = tc.alloc_tile_pool(name="osb", bufs=2)
        ppool = tc.alloc_tile_pool(name="psum", bufs=3, space="PSUM")
        oppool = tc.alloc_tile_pool(name="psum_o", bufs=2, space="PSUM")

        w_tiles = [w1_t, w2_t, w3_t]
        h_live = {}     # (chunk, li) -> produced tile (li 0 == x input)
        pso_live = {}   # chunk-pair -> psum_o tile
        gpk = {}        # li -> current packed gauss tile
        spk = {}        # li -> current packed sin tile
        gst = {li: {"blk": 0, "chunks": []} for li in range(3)}
        sst = {li: {"blk": 0, "chunks": []} for li in range(3)}

        hdt = F16 if use_fp32r else F32

        def emit_load(c):
            x_t = xpool.tile([G * N_IN, chunk], F16, tag="x")
            nc.sync.dma_start(
                out=x_t[:], in_=xT[:, c * chunk:(c + 1) * chunk])
            h_live[(c, 0)] = x_t

        def emit_main(c, li):
            """Main matmuls + full-height tanh pass + class gathers."""
            h_prev = h_live.pop((c, li))
            ng, ns = ngauss[li], nsin[li]
            kdim = G * N_IN if li == 0 else 128
            ps = ppool.tile([128, chunk], F32, tag="pre")
            wt = w_tiles[li]
            for hh in range(nhalf):
                sl = slice(hh * MM_N, (hh + 1) * MM_N)
                nc.tensor.matmul(
                    ps[:, sl],
                    wt[0:kdim, :],
                    h_prev[0:kdim, sl],
                    start=True, stop=True,
                )
            h = hpool.tile([128, chunk], hdt, tag=f"h{li}")
            cb = 8 * li
            # tanh-class pass over all 128 rows (junk on gauss/sin rows,
            # overwritten by the scatters)
            nc.scalar.activation(
                h[:], ps[:], mybir.ActivationFunctionType.Tanh,
                bias=col_t[:, cb + 1:cb + 2],
                scale=col_t[:, cb + 0:cb + 1],
            )
            # gather class rows (eps-encoded u+b) from h into packed tiles
            if ng > 0:
                st = gst[li]
                if st["blk"] == 0:
                    gpk[li] = gpool.tile([128, chunk], hdt, tag=f"gz{li}",
                                         name=f"gz{li}")
                eng = nc.sync if li < 2 else nc.gpsimd
                eng.dma_start(
                    out=gpk[li][st["blk"] * 4 * ng:
                                (st["blk"] + 1) * 4 * ng, :],
                    in_=h[0:4 * ng, :])
                st["blk"] += 1
                st["chunks"].append(c)
            if ns > 0:
                st = sst[li]
                if st["blk"] == 0:
                    spk[li] = spool.tile([128, chunk], hdt, tag=f"sz{li}",
                                         name=f"sz{li}")
                nc.gpsimd.dma_start(
                    out=spk[li][st["blk"] * 4 * ns:
                                (st["blk"] + 1) * 4 * ns, :],
                    in_=h[4 * ng:4 * (ng + ns), :])
                st["blk"] += 1
                st["chunks"].append(c)
            h_live[(c, li + 1)] = h

        def emit_gauss_pack(li):
            """Packed gauss chain for the accumulated group."""
            ng = ngauss[li]
            st = gst[li]
            nblk = st["blk"]
            R = 4 * ng * nblk
            gz = gpk[li]
            # y = Square((0.5/eps) * epsenc) = ((u+b)/2)^2 (in-place, fp16:
            # y is decoded so its magnitude is O(1) - fp16-safe)
            nc.scalar.activation(
                gz[0:R, :], gz[0:R, :],
                mybir.ActivationFunctionType.Square,
                scale=float(0.5 / ID_EPS),
            )
            # t = tanh(y)
            t_t = scpool.tile([128, chunk], F32, tag="gt")
            nc.scalar.activation(
                t_t[0:R, :], gz[0:R, :], mybir.ActivationFunctionType.Tanh)
            # den = 1 + t ;  r = 1/den ;  out = 2r - 1 = exp(-(z+b)^2/2)
            den_t = scpool.tile([128, chunk], F32, tag="gd")
            nc.vector.tensor_scalar(
                den_t[0:R, :], t_t[0:R, :], 1.0, None, mybir.AluOpType.add)
            rin_t = scpool.tile([128, chunk], F32, tag="gr")
            nc.vector.reciprocal_approx_fast(rin_t[0:R, :], den_t[0:R, :])
            g_r = rpool.tile([128, chunk], hdt, tag="go")
            nc.vector.tensor_scalar(
                g_r[0:R, :], rin_t[0:R, :], 2.0, -1.0,
                mybir.AluOpType.mult, mybir.AluOpType.add)
            # scatter back into each chunk's h rows [0 : 4ng)
            for blk, c in enumerate(st["chunks"]):
                nc.sync.dma_start(
                    out=h_live[(c, li + 1)][0:4 * ng, :],
                    in_=g_r[blk * 4 * ng:(blk + 1) * 4 * ng, :])
            st["blk"] = 0
            st["chunks"] = []

        def emit_sin_pack(li):
            """Packed sin chain for the accumulated group."""
            ng, ns = ngauss[li], nsin[li]
            st = sst[li]
            nblk = st["blk"]
            R = 4 * ns * nblk
            sz = spk[li]
            # wrap the eps-encoded u+b into [-eps*pi, eps*pi] (all linear,
            # so the wrap works in eps-space), then decode in Sin's scale.
            m_t = scpool.tile([128, chunk], F32, tag="sm")
            nc.vector.add_range_wrap(
                m_t[0:R, :], sz[0:R, :],
                0.0, float(ID_EPS) * PI, float(ID_EPS) * TWO_PI)
            s_r = rpool.tile([128, chunk], hdt, tag="so")
            nc.scalar.activation(
                s_r[0:R, :], m_t[0:R, :], mybir.ActivationFunctionType.Sin,
                scale=float(1.0 / ID_EPS))
            # scatter back into each chunk's h rows [4ng : 4(ng+ns))
            for blk, c in enumerate(st["chunks"]):
                nc.gpsimd.dma_start(
                    out=h_live[(c, li + 1)][4 * ng:4 * (ng + ns), :],
                    in_=s_r[blk * 4 * ns:(blk + 1) * 4 * ns, :])
            st["blk"] = 0
            st["chunks"] = []

        def emit_layer(c, li):
            emit_main(c, li)
            if ngauss[li] > 0 and ((c + goff[li]) % pfg[li] == pfg[li] - 1
                                   or c == nchunk - 1):
                emit_gauss_pack(li)
            if nsin[li] > 0 and ((c + soff[li]) % pfs[li] == pfs[li] - 1
                                 or c == nchunk - 1):
                emit_sin_pack(li)

        def emit_out(c):
            # output layer: quadrant-packed [12,512] matmuls
            h_prev = h_live.pop((c, 3))
            q0 = 2 * (c % 2)
            if q0 == 0:
                pso_live[c // 2] = oppool.tile([128, MM_N], F32, tag="preo",
                                               name="pso")
            pso = pso_live[c // 2]
            for hh in range(nhalf):
                q = q0 + hh
                nc.tensor.matmul(
                    pso[32 * q:32 * q + 32, :],
                    wo_t,
                    h_prev[:, hh * MM_N:(hh + 1) * MM_N],
                    start=True, stop=True,
                    tile_position=(0, 32 * q),
                )
            if q0 == 2:
                pso_live.pop(c // 2)
                osb = opool.tile([128, MM_N], F32, tag="osb")
                nc.scalar.activation(
                    osb[:], pso[:],
                    mybir.ActivationFunctionType.Tanh,
                    bias=col_t[:, 24:25],
                )
                base = (c - 1) * chunk
                for q in range(4):
                    nc.sync.dma_start(
                        out=yT[:, base + q * MM_N: base + (q + 1) * MM_N],
                        in_=osb[32 * q:32 * q + 12, :])

        # Software-pipelined emission with per-layer skew covering the packed
        # group latency: layer li+1 of chunk c is emitted only after layer
        # li's packed groups containing c have been emitted.
        assert nchunk % 2 == 0
        for t in range(nchunk + total_skew):
            if t < nchunk:
                emit_load(t)
                emit_layer(t, 0)
            if skew[1] <= t and t - skew[1] < nchunk:
                emit_layer(t - skew[1], 1)
            if skew[2] <= t and t - skew[2] < nchunk:
                emit_layer(t - skew[2], 2)
            if skew[3] <= t and t - skew[3] < nchunk:
                emit_out(t - skew[3])

        for p in (oppool, ppool, opool, rpool, scpool, spool,
                  gpool, hpool, xpool, cpool):
            p.release()

    nc.compile()
    return nc


_PROGRAM_CACHE = {}


def _get_program(ngauss, nsin, p_core=P_CORE, chunk=CHUNK, use_fp32r=True):
    key = (tuple(ngauss), tuple(nsin), p_core, chunk, use_fp32r)
    if key not in _PROGRAM_CACHE:
        _PROGRAM_CACHE[key] = _build_program(ngauss, nsin, p_core, chunk,
                                             use_fp32r=use_fp32r)
    return _PROGRAM_CACHE[key]


def make_in_maps(inputs, plan, p_core=P_CORE, n_cores=N_CORES):
    """Shard + transpose the pixel data; replicate constants."""
    x = np.ascontiguousarray(np.asarray(inputs["inputs"], dtype=np.float32))
    pg = p_core // G
    in_maps = []
    for core in range(n_cores):
        xc = x[core * p_core:(core + 1) * p_core]          # [p_core, 12]
        xg = xc.reshape(G, pg, N_IN)                        # [G, pg, 12]
        xT = np.ascontiguousarray(xg.transpose(0, 2, 1)
                                  .reshape(G * N_IN, pg)
                                  .astype(np.float16))      # [48, pg]
        wst = np.zeros((128, 416), dtype=np.float16)
        wst[0:G * N_IN, 0:128] = plan.lhsT[0]
        wst[:, 128:256] = plan.lhsT[1]
        wst[:, 256:384] = plan.lhsT[2]
        wst[:, 384:416] = plan.lhsT_out
        cst = np.zeros((128, 64), dtype=np.float32)
        cst[:, 0:32] = plan.colblk
        in_maps.append({"xT": xT, "wst": wst, "cst": cst})
    return in_maps


def assemble_output(results, p_core=P_CORE, n_cores=N_CORES):
    pg = p_core // G
    out = np.empty((p_core * n_cores, N_OUT), dtype=np.float32)
    for core in range(n_cores):
        yT = results[core]["yT"]                            # [12, pg]
        yc = yT.reshape(G, N_OUT, pg).transpose(0, 2, 1)    # [G, pg, 3]
        out[core * p_core:(core + 1) * p_core] = yc.reshape(p_core, N_OUT)
    return out


def make_plan(inputs):
    return _Plan(
        inputs["bias_in"], inputs["W1"], inputs["b1"], inputs["act1"],
        inputs["W2"], inputs["b2"], inputs["act2"],
        inputs["W3"], inputs["b3"], inputs["act3"],
        inputs["Wout"], inputs["bout"])


def run(inputs, trace=False, use_fp32r=True, **spmd_kwargs):
    plan = make_plan(inputs)
    nc = _get_program(plan.ngauss, plan.nsin, use_fp32r=use_fp32r)
    in_maps = make_in_maps(inputs, plan)
    res = run_bass_kernel_spmd(nc, in_maps, list(range(N_CORES)),
                               trace=trace, **spmd_kwargs)
    return assemble_output(res.results), res


def kernel(**inputs) -> np.ndarray:
    out, _ = run(inputs, trace=False)
    return out
